# revision 11
# baseline (speedup 1.0000x reference)
"""Trainium2 Bass kernel for nn_ARIMAModel (depthwise causal conv, 8 taps).

Math: reference output = window_part(x, ar) + window_part(x, ma); both windows
have k == 8 and window_part is linear in the weights, so

    out[b,n,i,f] = sum_{a=0}^{7} C[a,f] * x[b,n,i-8+a,f]   (i >= 8, else 0)
    C = ar_params + ma_params

Flattening each (b,n) sequence to a stream of S*F elements, the conv is a
banded linear map on blocks of the stream.  Data-parallel over 8 cores
(100 sequences each); no cross-core communication.

Per-core pipeline (mode "fp16", default):
  - host: cast x to fp16 (absmax rel err ~4e-4 vs fp32), pad, and store each
    chunk with even/odd 128-blocks de-interleaved so on-chip weight loads are
    unit-stride.
  - DMA-xbar transpose (HWDGE, 2-byte dtype) loads X^T tiles: partition =
    offset within 128-block, free = block index.
  - PE: for each 256-wide output block group, 3 matmuls with the data as the
    stationary operand and small banded weight matrices (built from C on
    host) as the moving operand; output lands in natural layout in PSUM
    (partition = 256-elem output block, free = offset) -> 1KB-contiguous
    DRAM rows on the way out.
  - PSUM -> SBUF copies alternate DVE/ACT; output DMA on the ACT HWDGE ring
    (transposes own the SP ring).
  - host: zero the first 8 stations of every sequence (conv warm-up region;
    also absorbs cross-sequence contamination in the flat stream).

Mode "bf16_split" (fallback, ~8.5e-6 rel err): x and C are each split into
bf16 hi+lo parts (x = xh + xl exactly to ~fp32); 6 matmul terms accumulate in
fp32 PSUM.  Twice the transpose traffic -> slower, but fp32-grade accuracy.
"""

import numpy as np
import ml_dtypes

BF16 = ml_dtypes.bfloat16

MODE = "fp16"                           # "fp16" | "bf16_split"

B, N, S, F = 4, 200, 4096, 8
K = 8
NCORES = 8
P = 128
SEQ_PER_CORE = B * N // NCORES          # 100
STREAM = SEQ_PER_CORE * S * F           # 3,276,800 elements per core
NBLK = STREAM // P                      # 25,600 blocks of 128

# fp16-mode tiling
CB = 5120                               # 128-blocks per chunk
OT_BANKS = 5                            # PSUM banks staged per output DMA

# bf16_split-mode tiling
SP_CB = 5120
SP_GROUP = 4
SP_OT_GROUPS = 5

_compiled = {}


# --------------------------------------------------------------------------
# fp16 mode
# --------------------------------------------------------------------------

def _make_nc_fp16(nblk, cb, ot_banks, n_cores):
    import concourse.mybir as mybir
    import concourse.tile as tile
    from concourse import bacc

    chunks = nblk // cb
    assert chunks * cb == nblk
    tw = cb + P                         # transposed cols per chunk (halo incl.)
    tw2 = tw // 2
    ncoarse = nblk // 2                 # 256-elem output blocks per core
    subtiles_per_chunk = cb // 256      # psum half-bank groups of 128 coarse
    banks_per_chunk = subtiles_per_chunk // 2
    otiles_per_chunk = banks_per_chunk // ot_banks
    assert otiles_per_chunk * ot_banks == banks_per_chunk
    ot_cols = ot_banks * 512            # output cols per staging tile

    nc = bacc.Bacc(
        "TRN2", target_bir_lowering=False, debug=False, num_devices=n_cores
    )
    f16 = mybir.dt.float16
    f32 = mybir.dt.float32

    # chunked + parity-deinterleaved input: x_d[c, j, :] rows are the chunk's
    # even 128-blocks then its odd 128-blocks (host lays this out)
    x_d = nc.dram_tensor("x16", [chunks, tw, P], f16, kind="ExternalInput")
    # weights: [W0 (256 cols, zero-padded) | Wm1 (64 cols)], stored
    # TRANSPOSED on host so the load can use the xbar-transpose path (keeps
    # phase 1 free of DMA-mode transitions)
    w_d = nc.dram_tensor("wts", [320, P], f16, kind="ExternalInput")
    y_d = nc.dram_tensor("y", [ncoarse, 256], f16, kind="ExternalOutput")

    def _ins(x):
        return getattr(x, "ins", x)

    with tile.TileContext(nc) as tc:
        from concourse.tile import add_dep_helper
        with tc.tile_pool(name="wpool", bufs=1) as wpool, \
             tc.tile_pool(name="xpool", bufs=chunks) as xpool, \
             tc.tile_pool(name="psum", bufs=8, space="PSUM") as psum, \
             tc.tile_pool(name="opool", bufs=chunks * otiles_per_chunk) as opool:
            W = wpool.tile([P, 320], f16)
            nc.sync.dma_start(out=W[:], in_=w_d[:], transpose=True)
            # Phase 1: all xbar transposes (SP ring), with PE matmuls and
            # PSUM->SBUF copies overlapping as chunks land.  Phase 2: output
            # DMAs, explicitly held until the LAST transpose completes -- the
            # HW xbar-mode bug forces Tile to serialize any transpose/copy
            # DMA pair, so interleaving them thrashes; one transition is free.
            tr_insts = []
            out_calls = []
            copy_flip = 0
            for c in range(chunks):
                xt = xpool.tile([P, tw], f16, tag="xt")
                tr = nc.sync.dma_start(out=xt[:], in_=x_d[c], transpose=True)
                tr_insts.append(_ins(tr))
                for ot in range(otiles_per_chunk):
                    otile = opool.tile([P, ot_cols], f16)
                    for g in range(ot_banks):
                        pt = psum.tile([P, 512], f32)
                        for half in range(2):
                            i = (ot * ot_banks + g) * 2 + half
                            A = i * P
                            o0 = half * 256
                            # S0 = odd blocks, S1/Sm1 = even blocks
                            s0 = xt[:, tw2 + A: tw2 + A + P]
                            s1 = xt[:, A + 1: A + 1 + P]
                            sm1 = xt[:, A: A + P]
                            nc.tensor.matmul(pt[:, o0: o0 + 256], s0,
                                             W[:, 0:256],
                                             start=True, stop=False)
                            nc.tensor.matmul(pt[:, o0 + 128: o0 + 256], s1,
                                             W[:, 0:128],
                                             start=False, stop=False)
                            nc.tensor.matmul(pt[:, o0: o0 + 64], sm1,
                                             W[:, 256:320],
                                             start=False, stop=True)
                        odst = otile[:, g * 512:(g + 1) * 512]
                        if copy_flip % 2 == 0:
                            nc.vector.tensor_copy(odst, pt[:])
                        else:
                            nc.scalar.copy(odst, pt[:])
                        copy_flip += 1
                    base = (c * banks_per_chunk + ot * ot_banks) * 256
                    out = nc.scalar.dma_start(
                        out=y_d[base: base + ot_banks * 256, :].rearrange(
                            "(m p) u -> p m u", p=P
                        ),
                        in_=otile[:].rearrange("p (m u) -> p m u", u=256),
                    )
                    out_calls.append(_ins(out))
            for o in out_calls:
                add_dep_helper(o, tr_insts[-1],
                               reason="hold output DMAs until last transpose")
    nc.compile()
    return nc


def _build_wts_fp16(Cmat):
    """[W0(256, zero-padded) | Wm1(64)] from C (8x8 fp32), in fp16.

    out[256C+u] = sum_lag C[8-lag, u%8] * xpad[256C+128 + (u-8*lag)]
      S0[v]  = xpad[256C+128+v]  -> W0[v, v+8lag]            (u = v+8lag)
      S1[v]  = xpad[256C+256+v]  -> W0[v, v+8lag] cols <128  (u = 128+v+8lag)
      Sm1[v] = xpad[256C+v]      -> Wm1[v, v-128+8lag]       (u = v-128+8lag)
    """
    C16 = Cmat.astype(np.float16).astype(np.float32)
    W0 = np.zeros((P, 256), np.float32)
    Wm1 = np.zeros((P, 64), np.float32)
    for v in range(P):
        f = v % 8
        for lag in range(1, 9):
            u = v + 8 * lag
            if u < 256:
                W0[v, u] = C16[8 - lag, f]
            um = v - 128 + 8 * lag
            if 0 <= um < 64:
                Wm1[v, um] = C16[8 - lag, f]
    return np.ascontiguousarray(
        np.concatenate([W0, Wm1], axis=1).T
    ).astype(np.float16)


def _prep_in_maps_fp16(x, ar_params, ma_params, n_cores, stream, nblk, cb):
    chunks = nblk // cb
    tw = cb + P
    padded = nblk + P
    Cmat = np.asarray(ar_params, np.float32) + np.asarray(ma_params, np.float32)
    wts = _build_wts_fp16(Cmat)
    xf = np.ascontiguousarray(np.asarray(x, dtype=np.float32)).reshape(
        n_cores, stream
    )
    pad = np.zeros((n_cores, padded, P), np.float16)
    pad[:, 1:1 + nblk, :] = xf.astype(np.float16).reshape(n_cores, nblk, P)
    # per-chunk parity de-interleave: even blocks then odd blocks
    perm = np.concatenate([np.arange(0, tw, 2), np.arange(1, tw, 2)])
    xd = np.empty((n_cores, chunks, tw, P), np.float16)
    for c in range(chunks):
        xd[:, c] = pad[:, c * cb: c * cb + tw, :][:, perm, :]
    return [
        {"x16": xd[core], "wts": wts} for core in range(n_cores)
    ]


# --------------------------------------------------------------------------
# bf16_split mode (fp32-grade fallback)
# --------------------------------------------------------------------------

def _make_nc_split(nblk, cb, group, ot_groups, n_cores):
    import concourse.mybir as mybir
    import concourse.tile as tile
    from concourse import bacc

    chunks = nblk // cb
    assert chunks * cb == nblk
    tw = cb + P
    tiles_per_chunk = cb // P
    groups_per_chunk = tiles_per_chunk // group
    otiles_per_chunk = groups_per_chunk // ot_groups
    assert otiles_per_chunk * ot_groups == groups_per_chunk
    ot_cols = ot_groups * group * P

    nc = bacc.Bacc(
        "TRN2", target_bir_lowering=False, debug=False, num_devices=n_cores
    )
    bf = mybir.dt.bfloat16
    f32 = mybir.dt.float32

    xh_d = nc.dram_tensor("xh", [nblk + P, P], bf, kind="ExternalInput")
    xl_d = nc.dram_tensor("xl", [nblk + P, P], bf, kind="ExternalInput")
    w_d = nc.dram_tensor("wts", [4, P, P], bf, kind="ExternalInput")
    y_d = nc.dram_tensor("y", [nblk, P], f32, kind="ExternalOutput")

    with tile.TileContext(nc) as tc:
        with tc.tile_pool(name="wpool", bufs=1) as wpool, \
             tc.tile_pool(name="xpool", bufs=2) as xpool, \
             tc.tile_pool(name="psum", bufs=6, space="PSUM") as psum, \
             tc.tile_pool(name="opool", bufs=2) as opool:
            W = wpool.tile([P, 4, P], bf)
            for j in range(4):
                nc.sync.dma_start(out=W[:, j, :], in_=w_d[j])
            for c in range(chunks):
                xh_t = xpool.tile([P, tw], bf, tag="xh_t")
                xl_t = xpool.tile([P, tw], bf, tag="xl_t")
                nc.sync.dma_start(
                    out=xh_t[:], in_=xh_d[c * cb: c * cb + tw, :],
                    transpose=True
                )
                nc.sync.dma_start(
                    out=xl_t[:], in_=xl_d[c * cb: c * cb + tw, :],
                    transpose=True
                )
                for ot in range(otiles_per_chunk):
                    otile = opool.tile([P, ot_cols], f16)
                    for g in range(ot_groups):
                        pt = psum.tile([P, group * P], f32)
                        for k in range(group):
                            t = (ot * ot_groups + g) * group + k
                            s = t * P
                            o = pt[:, k * P:(k + 1) * P]
                            nc.tensor.matmul(o, xh_t[:, s + 1: s + 1 + P],
                                             W[:, 0, :], start=True, stop=False)
                            nc.tensor.matmul(o, xh_t[:, s + 1: s + 1 + P],
                                             W[:, 2, :], start=False, stop=False)
                            nc.tensor.matmul(o, xh_t[:, s: s + P],
                                             W[:, 1, :], start=False, stop=False)
                            nc.tensor.matmul(o, xh_t[:, s: s + P],
                                             W[:, 3, :], start=False, stop=False)
                            nc.tensor.matmul(o, xl_t[:, s + 1: s + 1 + P],
                                             W[:, 0, :], start=False, stop=False)
                            nc.tensor.matmul(o, xl_t[:, s: s + P],
                                             W[:, 1, :], start=False, stop=True)
                        odst = otile[:, g * group * P:(g + 1) * group * P]
                        if g % 2 == 0:
                            nc.vector.tensor_copy(odst, pt[:])
                        else:
                            nc.scalar.copy(odst, pt[:])
                    base = c * cb + ot * ot_cols
                    nc.scalar.dma_start(
                        out=y_d[base: base + ot_cols, :].rearrange(
                            "(k p) u -> p k u", p=P
                        ),
                        in_=otile[:].rearrange("p (k u) -> p k u", u=P),
                    )
    nc.compile()
    return nc


def _mk_AB(Cm):
    A = np.zeros((P, P), np.float32)
    Bm = np.zeros((P, P), np.float32)
    for u in range(P):
        f = u % 8
        for lag in range(1, 9):
            coef = Cm[8 - lag, f]
            v = u - 8 * lag
            if v >= 0:
                A[v, u] = coef
            else:
                Bm[v + P, u] = coef
    return A, Bm


def _build_wts_split(Cmat):
    Chi = Cmat.astype(BF16).astype(np.float32)
    Clo = (Cmat - Chi).astype(BF16).astype(np.float32)
    Ahi, Bhi = _mk_AB(Chi)
    Alo, Blo = _mk_AB(Clo)
    return np.stack([Ahi, Bhi, Alo, Blo]).astype(BF16)


def _prep_in_maps_split(x, ar_params, ma_params, n_cores, stream, nblk):
    padded = nblk + P
    Cmat = np.asarray(ar_params, np.float32) + np.asarray(ma_params, np.float32)
    wts = _build_wts_split(Cmat)
    xf = np.ascontiguousarray(np.asarray(x, dtype=np.float32)).reshape(
        n_cores, stream
    )
    xh = xf.astype(BF16)
    xl = (xf - xh.astype(np.float32)).astype(BF16)
    ph = np.zeros((n_cores, padded * P), BF16)
    ph[:, P:P + stream] = xh
    pl = np.zeros((n_cores, padded * P), BF16)
    pl[:, P:P + stream] = xl
    return [
        {
            "xh": ph[c].reshape(padded, P),
            "xl": pl[c].reshape(padded, P),
            "wts": wts,
        }
        for c in range(n_cores)
    ]


# --------------------------------------------------------------------------
# driver
# --------------------------------------------------------------------------

def _get_nc(mode=MODE, **kw):
    if mode == "fp16":
        key = ("fp16", kw.get("nblk", NBLK), kw.get("cb", CB),
               kw.get("ot_banks", OT_BANKS), kw.get("n_cores", NCORES))
        if key not in _compiled:
            _compiled[key] = _make_nc_fp16(*key[1:])
    else:
        key = ("split", kw.get("nblk", NBLK), kw.get("cb", SP_CB),
               kw.get("group", SP_GROUP), kw.get("ot_groups", SP_OT_GROUPS),
               kw.get("n_cores", NCORES))
        if key not in _compiled:
            _compiled[key] = _make_nc_split(*key[1:])
    return _compiled[key]


def _run(x, ar_params, ma_params, trace=False, mode=MODE, **run_kwargs):
    from concourse.bass_utils import run_bass_kernel_spmd

    nc = _get_nc(mode)
    if mode == "fp16":
        in_maps = _prep_in_maps_fp16(x, ar_params, ma_params, NCORES, STREAM,
                                     NBLK, CB)
    else:
        in_maps = _prep_in_maps_split(x, ar_params, ma_params, NCORES, STREAM,
                                      NBLK)
    res = run_bass_kernel_spmd(
        nc, in_maps, core_ids=list(range(NCORES)), trace=trace, **run_kwargs
    )
    out = np.stack(
        [np.asarray(res.results[c]["y"], dtype=np.float32)
         for c in range(NCORES)]
    )
    out = out.reshape(B, N, S, F)
    out[:, :, :K, :] = 0.0
    return out, res


def kernel(x, ar_params, ma_params):
    out, _ = _run(x, ar_params, ma_params)
    return out


# revision 14
# speedup vs baseline: 1.0633x; 1.0633x over previous
"""Trainium2 Bass kernel for nn_ARIMAModel (depthwise causal conv, 8 taps).

Math: reference output = window_part(x, ar) + window_part(x, ma); both windows
have k == 8 and window_part is linear in the weights, so

    out[b,n,i,f] = sum_{a=0}^{7} C[a,f] * x[b,n,i-8+a,f]   (i >= 8, else 0)
    C = ar_params + ma_params

Flattening each (b,n) sequence to a stream of S*F elements, the conv is a
banded linear map on blocks of the stream.  Data-parallel over 8 cores
(100 sequences each); no cross-core communication.

Per-core pipeline (mode "fp16", default):
  - host: cast x to fp16 (absmax rel err ~4e-4 vs fp32), pad, and store each
    chunk with even/odd 128-blocks de-interleaved so on-chip weight loads are
    unit-stride.
  - DMA-xbar transpose (HWDGE, 2-byte dtype) loads X^T tiles: partition =
    offset within 128-block, free = block index.
  - PE: for each 256-wide output block group, 3 matmuls with the data as the
    stationary operand and small banded weight matrices (built from C on
    host) as the moving operand; output lands in natural layout in PSUM
    (partition = 256-elem output block, free = offset) -> 1KB-contiguous
    DRAM rows on the way out.
  - PSUM -> SBUF copies alternate DVE/ACT; output DMA on the ACT HWDGE ring
    (transposes own the SP ring).
  - host: zero the first 8 stations of every sequence (conv warm-up region;
    also absorbs cross-sequence contamination in the flat stream).

Mode "bf16_split" (fallback, ~8.5e-6 rel err): x and C are each split into
bf16 hi+lo parts (x = xh + xl exactly to ~fp32); 6 matmul terms accumulate in
fp32 PSUM.  Twice the transpose traffic -> slower, but fp32-grade accuracy.
"""

import numpy as np
import ml_dtypes

BF16 = ml_dtypes.bfloat16

MODE = "pe"                             # "pe" | "fp16" | "bf16_split"

B, N, S, F = 4, 200, 4096, 8
K = 8
NCORES = 8
P = 128
SEQ_PER_CORE = B * N // NCORES          # 100
STREAM = SEQ_PER_CORE * S * F           # 3,276,800 elements per core
NBLK = STREAM // P                      # 25,600 blocks of 128

# fp16-mode tiling
CB = 5120                               # 128-blocks per chunk
OT_BANKS = 5                            # PSUM banks staged per output DMA

# bf16_split-mode tiling
SP_CB = 5120
SP_GROUP = 4
SP_OT_GROUPS = 5

_compiled = {}


# --------------------------------------------------------------------------
# fp16 mode
# --------------------------------------------------------------------------

def _make_nc_fp16(nblk, cb, ot_banks, n_cores):
    import concourse.mybir as mybir
    import concourse.tile as tile
    from concourse import bacc

    chunks = nblk // cb
    assert chunks * cb == nblk
    tw = cb + P                         # transposed cols per chunk (halo incl.)
    tw2 = tw // 2
    ncoarse = nblk // 2                 # 256-elem output blocks per core
    subtiles_per_chunk = cb // 256      # psum half-bank groups of 128 coarse
    banks_per_chunk = subtiles_per_chunk // 2
    otiles_per_chunk = banks_per_chunk // ot_banks
    assert otiles_per_chunk * ot_banks == banks_per_chunk
    ot_cols = ot_banks * 512            # output cols per staging tile

    nc = bacc.Bacc(
        "TRN2", target_bir_lowering=False, debug=False, num_devices=n_cores
    )
    f16 = mybir.dt.float16
    f32 = mybir.dt.float32

    # chunked + parity-deinterleaved input: x_d[c, j, :] rows are the chunk's
    # even 128-blocks then its odd 128-blocks (host lays this out)
    x_d = nc.dram_tensor("x16", [chunks, tw, P], f16, kind="ExternalInput")
    # weights: [W0 (256 cols, zero-padded) | Wm1 (64 cols)], stored
    # TRANSPOSED on host so the load can use the xbar-transpose path (keeps
    # phase 1 free of DMA-mode transitions)
    w_d = nc.dram_tensor("wts", [320, P], f16, kind="ExternalInput")
    y_d = nc.dram_tensor("y", [ncoarse, 256], f16, kind="ExternalOutput")

    def _ins(x):
        return getattr(x, "ins", x)

    with tile.TileContext(nc) as tc:
        from concourse.tile import add_dep_helper
        with tc.tile_pool(name="wpool", bufs=1) as wpool, \
             tc.tile_pool(name="xpool", bufs=chunks) as xpool, \
             tc.tile_pool(name="psum", bufs=8, space="PSUM") as psum, \
             tc.tile_pool(name="opool", bufs=chunks * otiles_per_chunk) as opool:
            W = wpool.tile([P, 320], f16)
            nc.sync.dma_start(out=W[:], in_=w_d[:], transpose=True)
            # Phase 1: all xbar transposes (SP ring), with PE matmuls and
            # PSUM->SBUF copies overlapping as chunks land.  Phase 2: output
            # DMAs, explicitly held until the LAST transpose completes -- the
            # HW xbar-mode bug forces Tile to serialize any transpose/copy
            # DMA pair, so interleaving them thrashes; one transition is free.
            tr_insts = []
            out_calls = []
            copy_flip = 0
            for c in range(chunks):
                xt = xpool.tile([P, tw], f16, tag="xt")
                tr = nc.sync.dma_start(out=xt[:], in_=x_d[c], transpose=True)
                tr_insts.append(_ins(tr))
                for ot in range(otiles_per_chunk):
                    otile = opool.tile([P, ot_cols], f16)
                    for g in range(ot_banks):
                        pt = psum.tile([P, 512], f32)
                        for half in range(2):
                            i = (ot * ot_banks + g) * 2 + half
                            A = i * P
                            o0 = half * 256
                            # S0 = odd blocks, S1/Sm1 = even blocks
                            s0 = xt[:, tw2 + A: tw2 + A + P]
                            s1 = xt[:, A + 1: A + 1 + P]
                            sm1 = xt[:, A: A + P]
                            nc.tensor.matmul(pt[:, o0: o0 + 256], s0,
                                             W[:, 0:256],
                                             start=True, stop=False)
                            nc.tensor.matmul(pt[:, o0 + 128: o0 + 256], s1,
                                             W[:, 0:128],
                                             start=False, stop=False)
                            nc.tensor.matmul(pt[:, o0: o0 + 64], sm1,
                                             W[:, 256:320],
                                             start=False, stop=True)
                        odst = otile[:, g * 512:(g + 1) * 512]
                        if copy_flip % 2 == 0:
                            nc.vector.tensor_copy(odst, pt[:])
                        else:
                            nc.scalar.copy(odst, pt[:])
                        copy_flip += 1
                    base = (c * banks_per_chunk + ot * ot_banks) * 256
                    out = nc.scalar.dma_start(
                        out=y_d[base: base + ot_banks * 256, :].rearrange(
                            "(m p) u -> p m u", p=P
                        ),
                        in_=otile[:].rearrange("p (m u) -> p m u", u=256),
                    )
                    out_calls.append(_ins(out))
            for o in out_calls:
                add_dep_helper(o, tr_insts[-1],
                               reason="hold output DMAs until last transpose")
    nc.compile()
    return nc


def _build_wts_fp16(Cmat, transposed=True):
    """[W0(256, zero-padded) | Wm1(64)] from C (8x8 fp32), in fp16.

    out[256C+u] = sum_lag C[8-lag, u%8] * xpad[256C+128 + (u-8*lag)]
      S0[v]  = xpad[256C+128+v]  -> W0[v, v+8lag]            (u = v+8lag)
      S1[v]  = xpad[256C+256+v]  -> W0[v, v+8lag] cols <128  (u = 128+v+8lag)
      Sm1[v] = xpad[256C+v]      -> Wm1[v, v-128+8lag]       (u = v-128+8lag)
    """
    C16 = Cmat.astype(np.float16).astype(np.float32)
    W0 = np.zeros((P, 256), np.float32)
    Wm1 = np.zeros((P, 64), np.float32)
    for v in range(P):
        f = v % 8
        for lag in range(1, 9):
            u = v + 8 * lag
            if u < 256:
                W0[v, u] = C16[8 - lag, f]
            um = v - 128 + 8 * lag
            if 0 <= um < 64:
                Wm1[v, um] = C16[8 - lag, f]
    W = np.concatenate([W0, Wm1], axis=1)
    if transposed:
        W = np.ascontiguousarray(W.T)
    return W.astype(np.float16)


def _prep_in_maps_fp16(x, ar_params, ma_params, n_cores, stream, nblk, cb):
    chunks = nblk // cb
    tw = cb + P
    padded = nblk + P
    Cmat = np.asarray(ar_params, np.float32) + np.asarray(ma_params, np.float32)
    wts = _build_wts_fp16(Cmat)
    xf = np.ascontiguousarray(np.asarray(x, dtype=np.float32)).reshape(
        n_cores, stream
    )
    pad = np.zeros((n_cores, padded, P), np.float16)
    pad[:, 1:1 + nblk, :] = xf.astype(np.float16).reshape(n_cores, nblk, P)
    # per-chunk parity de-interleave: even blocks then odd blocks
    perm = np.concatenate([np.arange(0, tw, 2), np.arange(1, tw, 2)])
    xd = np.empty((n_cores, chunks, tw, P), np.float16)
    for c in range(chunks):
        xd[:, c] = pad[:, c * cb: c * cb + tw, :][:, perm, :]
    return [
        {"x16": xd[core], "wts": wts} for core in range(n_cores)
    ]


# --------------------------------------------------------------------------
# bf16_split mode (fp32-grade fallback)
# --------------------------------------------------------------------------

def _make_nc_split(nblk, cb, group, ot_groups, n_cores):
    import concourse.mybir as mybir
    import concourse.tile as tile
    from concourse import bacc

    chunks = nblk // cb
    assert chunks * cb == nblk
    tw = cb + P
    tiles_per_chunk = cb // P
    groups_per_chunk = tiles_per_chunk // group
    otiles_per_chunk = groups_per_chunk // ot_groups
    assert otiles_per_chunk * ot_groups == groups_per_chunk
    ot_cols = ot_groups * group * P

    nc = bacc.Bacc(
        "TRN2", target_bir_lowering=False, debug=False, num_devices=n_cores
    )
    bf = mybir.dt.bfloat16
    f32 = mybir.dt.float32

    xh_d = nc.dram_tensor("xh", [nblk + P, P], bf, kind="ExternalInput")
    xl_d = nc.dram_tensor("xl", [nblk + P, P], bf, kind="ExternalInput")
    w_d = nc.dram_tensor("wts", [4, P, P], bf, kind="ExternalInput")
    y_d = nc.dram_tensor("y", [nblk, P], f32, kind="ExternalOutput")

    with tile.TileContext(nc) as tc:
        with tc.tile_pool(name="wpool", bufs=1) as wpool, \
             tc.tile_pool(name="xpool", bufs=2) as xpool, \
             tc.tile_pool(name="psum", bufs=6, space="PSUM") as psum, \
             tc.tile_pool(name="opool", bufs=2) as opool:
            W = wpool.tile([P, 4, P], bf)
            for j in range(4):
                nc.sync.dma_start(out=W[:, j, :], in_=w_d[j])
            for c in range(chunks):
                xh_t = xpool.tile([P, tw], bf, tag="xh_t")
                xl_t = xpool.tile([P, tw], bf, tag="xl_t")
                nc.sync.dma_start(
                    out=xh_t[:], in_=xh_d[c * cb: c * cb + tw, :],
                    transpose=True
                )
                nc.sync.dma_start(
                    out=xl_t[:], in_=xl_d[c * cb: c * cb + tw, :],
                    transpose=True
                )
                for ot in range(otiles_per_chunk):
                    otile = opool.tile([P, ot_cols], f16)
                    for g in range(ot_groups):
                        pt = psum.tile([P, group * P], f32)
                        for k in range(group):
                            t = (ot * ot_groups + g) * group + k
                            s = t * P
                            o = pt[:, k * P:(k + 1) * P]
                            nc.tensor.matmul(o, xh_t[:, s + 1: s + 1 + P],
                                             W[:, 0, :], start=True, stop=False)
                            nc.tensor.matmul(o, xh_t[:, s + 1: s + 1 + P],
                                             W[:, 2, :], start=False, stop=False)
                            nc.tensor.matmul(o, xh_t[:, s: s + P],
                                             W[:, 1, :], start=False, stop=False)
                            nc.tensor.matmul(o, xh_t[:, s: s + P],
                                             W[:, 3, :], start=False, stop=False)
                            nc.tensor.matmul(o, xl_t[:, s + 1: s + 1 + P],
                                             W[:, 0, :], start=False, stop=False)
                            nc.tensor.matmul(o, xl_t[:, s: s + P],
                                             W[:, 1, :], start=False, stop=True)
                        odst = otile[:, g * group * P:(g + 1) * group * P]
                        if g % 2 == 0:
                            nc.vector.tensor_copy(odst, pt[:])
                        else:
                            nc.scalar.copy(odst, pt[:])
                    base = c * cb + ot * ot_cols
                    nc.scalar.dma_start(
                        out=y_d[base: base + ot_cols, :].rearrange(
                            "(k p) u -> p k u", p=P
                        ),
                        in_=otile[:].rearrange("p (k u) -> p k u", u=P),
                    )
    nc.compile()
    return nc


def _mk_AB(Cm):
    A = np.zeros((P, P), np.float32)
    Bm = np.zeros((P, P), np.float32)
    for u in range(P):
        f = u % 8
        for lag in range(1, 9):
            coef = Cm[8 - lag, f]
            v = u - 8 * lag
            if v >= 0:
                A[v, u] = coef
            else:
                Bm[v + P, u] = coef
    return A, Bm


def _build_wts_split(Cmat):
    Chi = Cmat.astype(BF16).astype(np.float32)
    Clo = (Cmat - Chi).astype(BF16).astype(np.float32)
    Ahi, Bhi = _mk_AB(Chi)
    Alo, Blo = _mk_AB(Clo)
    return np.stack([Ahi, Bhi, Alo, Blo]).astype(BF16)


def _prep_in_maps_split(x, ar_params, ma_params, n_cores, stream, nblk):
    padded = nblk + P
    Cmat = np.asarray(ar_params, np.float32) + np.asarray(ma_params, np.float32)
    wts = _build_wts_split(Cmat)
    xf = np.ascontiguousarray(np.asarray(x, dtype=np.float32)).reshape(
        n_cores, stream
    )
    xh = xf.astype(BF16)
    xl = (xf - xh.astype(np.float32)).astype(BF16)
    ph = np.zeros((n_cores, padded * P), BF16)
    ph[:, P:P + stream] = xh
    pl = np.zeros((n_cores, padded * P), BF16)
    pl[:, P:P + stream] = xl
    return [
        {
            "xh": ph[c].reshape(padded, P),
            "xl": pl[c].reshape(padded, P),
            "wts": wts,
        }
        for c in range(n_cores)
    ]


# --------------------------------------------------------------------------
# pe mode: no DMA-xbar at all.  Plain big-descriptor loads (overlap the
# output stream freely), PE transpose-mode matmuls build the X^T tiles
# on-chip, and the span-major layout makes output rows ~10KB contiguous.
# --------------------------------------------------------------------------

def _make_nc_pe(L, load_cols, g_stage, n_cores):
    import concourse.mybir as mybir
    import concourse.tile as tile
    from concourse import bacc

    NJ = L // P + 1                      # 128-col transpose tiles (incl halo)
    NG = L // 256                        # 256-elem output groups per partition
    assert (NJ - 1) % (load_cols // P) == 0
    nloads = (NJ - 1) // (load_cols // P)
    assert NG % g_stage == 0
    notiles = NG // g_stage

    nc = bacc.Bacc(
        "TRN2", target_bir_lowering=False, debug=False, num_devices=n_cores
    )
    f16 = mybir.dt.float16
    f32 = mybir.dt.float32

    x_d = nc.dram_tensor("xin", [P, L + P], f16, kind="ExternalInput")
    w_d = nc.dram_tensor("wts", [P, 320], f16, kind="ExternalInput")
    id_d = nc.dram_tensor("ident", [P, P], f16, kind="ExternalInput")
    y_d = nc.dram_tensor("y", [P, L], f16, kind="ExternalOutput")

    jgrp = load_cols // P                # transpose tiles per load group

    with tile.TileContext(nc) as tc:
        with tc.tile_pool(name="wpool", bufs=1) as wpool, \
             tc.tile_pool(name="xpool", bufs=nloads + 1) as xpool, \
             tc.tile_pool(name="tq", bufs=4) as tqpool, \
             tc.tile_pool(name="pst", bufs=2, space="PSUM") as pst, \
             tc.tile_pool(name="pso", bufs=4, space="PSUM") as pso, \
             tc.tile_pool(name="opool", bufs=2) as opool:
            W = wpool.tile([P, 320], f16, tag="w")
            ident = wpool.tile([P, P], f16, tag="ident")
            nc.sync.dma_start(out=W[:], in_=w_d[:])
            nc.sync.dma_start(out=ident[:], in_=id_d[:])
            xts = []
            for gl in range(nloads):
                xt = xpool.tile([P, load_cols], f16, tag="xin")
                nc.sync.dma_start(
                    out=xt[:], in_=x_d[:, gl * load_cols:(gl + 1) * load_cols]
                )
                xts.append(xt)
            xhalo = xpool.tile([P, P], f16, tag="xhalo")
            nc.sync.dma_start(out=xhalo[:], in_=x_d[:, L:])

            def src_of(j):
                if j == NJ - 1:
                    return xhalo[:, 0:P]
                return xts[j // jgrp][:, (j % jgrp) * P:(j % jgrp + 1) * P]

            # T_j (transposed tiles) are built in quads: 4 PE transposes into
            # one f16 PSUM bank, one copy out to SBUF.
            tq_tiles = {}                # quad index -> sbuf tile
            def t_of(j):
                q, off = j // 4, (j % 4) * P
                return tq_tiles[q][:, off: off + P]

            nquads = (NJ + 3) // 4
            copy_flip = 0
            g_next = 0
            otile = None
            for q in range(nquads):
                ptile = pst.tile([P, 512], f16)
                j_hi = min(4 * q + 4, NJ)
                for j in range(4 * q, j_hi):
                    nc.tensor.transpose(
                        ptile[:, (j % 4) * P:(j % 4 + 1) * P], src_of(j),
                        ident[:]
                    )
                tqt = tqpool.tile([P, 512], f16, tag="tq")
                nc.vector.tensor_copy(tqt[:], ptile[:])
                tq_tiles[q] = tqt
                # emit conv groups whose inputs are now all transposed
                while g_next < NG and 2 * g_next + 2 < j_hi:
                    g = g_next
                    if g % 2 == 0:
                        po = pso.tile([P, 512], f32)
                    o0 = (g % 2) * 256
                    nc.tensor.matmul(po[:, o0: o0 + 256], t_of(2 * g + 1),
                                     W[:, 0:256], start=True, stop=False)
                    nc.tensor.matmul(po[:, o0 + 128: o0 + 256], t_of(2 * g + 2),
                                     W[:, 0:128], start=False, stop=False)
                    nc.tensor.matmul(po[:, o0: o0 + 64], t_of(2 * g),
                                     W[:, 256:320], start=False, stop=True)
                    if g % 2 == 1:
                        if g // 2 % (g_stage // 2) == 0:
                            otile = opool.tile([P, g_stage * 256], f16,
                                               tag="ot")
                        oc = (g // 2 % (g_stage // 2)) * 512
                        odst = otile[:, oc: oc + 512]
                        if copy_flip % 2 == 0:
                            nc.vector.tensor_copy(odst, po[:])
                        else:
                            nc.scalar.copy(odst, po[:])
                        copy_flip += 1
                        if (g + 1) % g_stage == 0:
                            o_idx = g // g_stage
                            nc.scalar.dma_start(
                                out=y_d[:, o_idx * g_stage * 256:
                                        (o_idx + 1) * g_stage * 256],
                                in_=otile[:],
                            )
                    g_next += 1
    nc.compile()
    return nc


def _prep_in_maps_pe(x, ar_params, ma_params, n_cores, stream, L):
    Cmat = np.asarray(ar_params, np.float32) + np.asarray(ma_params, np.float32)
    wts = _build_wts_fp16(Cmat, transposed=False)
    xf = np.ascontiguousarray(np.asarray(x, dtype=np.float32)).reshape(
        n_cores, stream
    )
    xpad = np.zeros((n_cores, P + stream), np.float16)
    xpad[:, P:] = xf.astype(np.float16)
    ident = np.eye(P, dtype=np.float16)
    maps = []
    for c in range(n_cores):
        win = np.lib.stride_tricks.as_strided(
            xpad[c], (P, L + P), (L * 2, 2)
        )
        maps.append({
            "xin": np.ascontiguousarray(win),
            "wts": wts,
            "ident": ident,
        })
    return maps


# --------------------------------------------------------------------------
# driver
# --------------------------------------------------------------------------

def _get_nc(mode=MODE, **kw):
    if mode == "pe":
        key = ("pe", kw.get("L", STREAM // P), kw.get("load_cols", 3200),
               kw.get("g_stage", 20), kw.get("n_cores", NCORES))
        if key not in _compiled:
            _compiled[key] = _make_nc_pe(*key[1:])
        return _compiled[key]
    if mode == "fp16":
        key = ("fp16", kw.get("nblk", NBLK), kw.get("cb", CB),
               kw.get("ot_banks", OT_BANKS), kw.get("n_cores", NCORES))
        if key not in _compiled:
            _compiled[key] = _make_nc_fp16(*key[1:])
    else:
        key = ("split", kw.get("nblk", NBLK), kw.get("cb", SP_CB),
               kw.get("group", SP_GROUP), kw.get("ot_groups", SP_OT_GROUPS),
               kw.get("n_cores", NCORES))
        if key not in _compiled:
            _compiled[key] = _make_nc_split(*key[1:])
    return _compiled[key]


def _run(x, ar_params, ma_params, trace=False, mode=MODE, **run_kwargs):
    from concourse.bass_utils import run_bass_kernel_spmd

    nc = _get_nc(mode)
    if mode == "pe":
        in_maps = _prep_in_maps_pe(x, ar_params, ma_params, NCORES, STREAM,
                                   STREAM // P)
    elif mode == "fp16":
        in_maps = _prep_in_maps_fp16(x, ar_params, ma_params, NCORES, STREAM,
                                     NBLK, CB)
    else:
        in_maps = _prep_in_maps_split(x, ar_params, ma_params, NCORES, STREAM,
                                      NBLK)
    res = run_bass_kernel_spmd(
        nc, in_maps, core_ids=list(range(NCORES)), trace=trace, **run_kwargs
    )
    out = np.stack(
        [np.asarray(res.results[c]["y"], dtype=np.float32)
         for c in range(NCORES)]
    )
    out = out.reshape(B, N, S, F)
    out[:, :, :K, :] = 0.0
    return out, res


def kernel(x, ar_params, ma_params):
    out, _ = _run(x, ar_params, ma_params)
    return out
